# revision 1
# baseline (speedup 1.0000x reference)
"""CAM (channel attention) module kernel for Trainium2, 8-core data-parallel.

Reference computation (per sample b):
    q = conv2d(x, Wq, stride2, 2x2) -> [C, 4096]
    k = conv2d(x, Wk, stride2, 2x2) -> [C, 4096]
    v = conv2d(x, Wv, 1x1)          -> [C, 16384]
    E = q @ k^T                      [C, C]
    att = softmax(rowmax(E) - E)   (== softmin over rows)
    out = att @ v -> [C, H, W]

Kernel strategy (one sample per NeuronCore, B=8 over 8 cores):
  - The softmax here is extremely peaked (energy entries span +-200), so
    energy errors are amplified exponentially: q/k need ~18+ mantissa
    bits, which rules out bf16 and single-pass fp32r (12-bit) for the
    convs. Native fp32 matmul costs 2 half-rate passes (4 cyc/row).
  - Measured on HW: 4-byte moving operands (fp32 AND fp32r) stream at
    2 cyc/row; bf16 streams at 1 cyc/row. So the cheapest precise
    scheme is split-bf16: x = xh + xl with xh = bf16(x), xl =
    bf16(x - xh) (~16-bit combined); same for the conv weights.
    conv = Wh@xh + Wh@xl + Wl@xh: 3 full-rate bf16 passes (3 cyc/row
    vs fp32's 4) with ~6.5e-4 worst-case output impact.
  - conv produces q in [c, n] layout via strided im2col APs from the
    resident xr/xl2 tiles (4 accumulating taps x 3 passes per band),
    then PE-transposes to [n, c] chunks for the energy contraction.
  - energy e = q k^T in native fp32 (exact; N=128 makes fp32r slow
    there anyway), accumulated over 32 chunk matmuls in one PSUM bank.
  - softmax via one DVE row-min + one ScalarE exp (bias=rowmin,
    scale=-1) with fused accumulated row-sum.
  - out = att @ (Wv x + bv) == (att Wv) @ x + (att bv) 1^T: computes
    M^T = Wv^T att^T on PE ([128,128]), splits M the same way, and
    runs out = Mh@xh + Mh@xl + Ml@xh against the resident split-x
    tiles (3 bf16 passes, ~1e-5 error; reuses the conv's xh/xl).
"""

import numpy as np

B, C, H, W = 8, 128, 128, 128
HW = H * W           # 16384
N_CORES = 8
NB = 8               # number of H-bands (16 input rows each)
BAND = HW // NB      # 2048 x columns per band
QN = (H // 2) * (W // 2)  # 4096 conv output positions
QCHUNK = QN // NB    # 512 conv outputs per band

_CACHE = {}


def _build_program(with_qk_bias: bool, with_v_bias: bool):
    import concourse.tile as tile
    from concourse import bacc, mybir
    from concourse.masks import make_identity

    f32 = mybir.dt.float32
    bf16 = mybir.dt.bfloat16
    Ident = mybir.ActivationFunctionType.Identity
    CopyF = mybir.ActivationFunctionType.Copy
    nc = bacc.Bacc(
        "TRN2", target_bir_lowering=False, debug=False, num_devices=N_CORES)

    x_d = nc.declare_dram_parameter("x", [C, HW], f32, isOutput=False)
    wqk_d = nc.declare_dram_parameter("wqk", [C, 8 * C], f32, isOutput=False)
    wv_d = nc.declare_dram_parameter("wv", [C, C], f32, isOutput=False)
    if with_qk_bias:
        bq_d = nc.declare_dram_parameter("bq", [C, 1], f32, isOutput=False)
        bk_d = nc.declare_dram_parameter("bk", [C, 1], f32, isOutput=False)
    if with_v_bias:
        bv_d = nc.declare_dram_parameter("bv", [C, 1], f32, isOutput=False)
    out_d = nc.declare_dram_parameter("out", [C, HW], f32, isOutput=True)

    with tile.TileContext(nc) as tc:
        with (
            tc.tile_pool(name="const", bufs=1) as const,
            tc.tile_pool(name="xstage", bufs=3) as xstage,
            tc.tile_pool(name="xrp", bufs=1) as xrp,
            tc.tile_pool(name="xl2p", bufs=3) as xl2p,
            tc.tile_pool(name="qkT", bufs=1) as qkT,
            tc.tile_pool(name="stage", bufs=3) as stage,
            tc.tile_pool(name="oout", bufs=3) as oout,
            tc.tile_pool(name="small", bufs=2) as small,
            tc.tile_pool(name="pacc", bufs=4, space="PSUM") as pacc,
            tc.tile_pool(name="ptp", bufs=2, space="PSUM") as ptp,
            tc.tile_pool(name="psm", bufs=1, space="PSUM") as psm,
        ):
            ident = const.tile([128, 128], f32, tag="ident")
            make_identity(nc, ident)

            # Startup: per-core DMA bandwidth (~350GB/s) is ONE shared
            # resource; concurrent queues just split it and delay the
            # critical band-0 input. Serialize ALL input loads on the sync
            # queue in consumption order so each transfer gets full
            # bandwidth: x0 first, then the conv weights, then the
            # remaining bands (each lands well before its conv slot).
            x_sb = []
            xh_sb = [xrp.tile([C, BAND], bf16, tag=f"xh{j}", name=f"xh{j}")
                     for j in range(NB)]
            xl_sb = [xrp.tile([C, BAND], bf16, tag=f"xl{j}", name=f"xl{j}")
                     for j in range(NB)]
            x0 = xstage.tile([C, BAND], f32, tag="x")
            nc.sync.dma_start(out=x0, in_=x_d[:, 0:BAND])
            x_sb.append(x0)
            wqk_sb = const.tile([C, 8 * C], f32, tag="wqk")
            nc.sync.dma_start(out=wqk_sb, in_=wqk_d[:, :])
            wqT_sb = wqk_sb[:, 0:4 * C]
            wkT_sb = wqk_sb[:, 4 * C:8 * C]
            if with_qk_bias:
                bq_sb = const.tile([C, 1], f32, tag="bq")
                nc.sync.dma_start(out=bq_sb, in_=bq_d[:, :])
                bk_sb = const.tile([C, 1], f32, tag="bk")
                nc.sync.dma_start(out=bk_sb, in_=bk_d[:, :])
            for j in range(1, NB):
                t = xstage.tile([C, BAND], f32, tag="x", name=f"x{j}")
                nc.sync.dma_start(out=t, in_=x_d[:, j * BAND:(j + 1) * BAND])
                x_sb.append(t)
            wv_sb = const.tile([C, C], f32, tag="wv")
            nc.sync.dma_start(out=wv_sb, in_=wv_d[:, :])
            if with_v_bias:
                bv_sb = const.tile([C, 1], f32, tag="bv")
                nc.sync.dma_start(out=bv_sb, in_=bv_d[:, :])

            # split the conv weights into bf16 hi/lo; keep DVE free for the
            # x band-0 quarter splits (hi cast on ScalarE, lo sub on GpSimd)
            def split_w(w_f32, name):
                hi = const.tile([C, 4 * C], bf16, tag=f"{name}hi")
                nc.scalar.activation(out=hi, in_=w_f32, func=CopyF,
                                     bias=0.0, scale=1.0)
                lo = const.tile([C, 4 * C], bf16, tag=f"{name}lo")
                nc.gpsimd.tensor_tensor(
                    out=lo, in0=w_f32, in1=hi,
                    op=mybir.AluOpType.subtract)
                return hi, lo

            wqh, wql = split_w(wqT_sb, "wq")
            wkh, wkl = split_w(wkT_sb, "wk")

            qT = [qkT.tile([128, QCHUNK], f32, tag=f"qT{j}", name=f"qT{j}")
                  for j in range(NB)]
            kT = [qkT.tile([128, QCHUNK], f32, tag=f"kT{j}", name=f"kT{j}")
                  for j in range(NB)]

            def conv_band(j, wh, wl, xr_v, xl_v, bias_sb):
                """12 accumulating matmuls -> PSUM [128, 512] (q for band j),
                returns the psum tile."""
                acc = pacc.tile([128, QCHUNK], f32, tag="acc")
                n_mm = 0
                for ab in range(4):
                    a, bb = ab // 2, ab % 2
                    for lhsT, rhs in (
                        (wh[:, ab * C:(ab + 1) * C], xr_v[:, :, a, :, bb]),
                        (wh[:, ab * C:(ab + 1) * C], xl_v[:, :, a, :, bb]),
                        (wl[:, ab * C:(ab + 1) * C], xr_v[:, :, a, :, bb]),
                    ):
                        nc.tensor.matmul(acc, lhsT=lhsT, rhs=rhs,
                                         start=(n_mm == 0), stop=(n_mm == 11))
                        n_mm += 1
                return acc

            def emit_transposes(j, qc, kc):
                for T_out, src in ((qT[j], qc), (kT[j], kc)):
                    tp = ptp.tile([128, QCHUNK], f32, tag="tp")
                    for t in range(4):
                        nc.tensor.transpose(
                            tp[:, t * 128:(t + 1) * 128],
                            src[:, t * 128:(t + 1) * 128], ident)
                    nc.scalar.activation(out=T_out, in_=tp, func=CopyF,
                                         bias=0.0, scale=1.0)

            # energy accumulator lives across the whole conv phase: energy
            # chunk matmuls are interleaved into the conv stream so their
            # weight loads hide under conv matmuls and the PE never sits in
            # a low-duty phase (which would re-throttle the HAM clock gate).
            E = psm.tile([128, 128], f32, tag="E")
            e_idx = [0]

            def emit_energy(j):
                for t in range(4):
                    nc.tensor.matmul(
                        E,
                        lhsT=qT[j][:, t * 128:(t + 1) * 128],
                        rhs=kT[j][:, t * 128:(t + 1) * 128],
                        start=(e_idx[0] == 0), stop=(e_idx[0] == NB * 4 - 1))
                    e_idx[0] += 1

            def split_band(j):
                xh_t, xl_t = xh_sb[j], xl_sb[j]
                nc.vector.tensor_copy(xh_t, x_sb[j])
                nc.vector.tensor_tensor(
                    out=xl_t, in0=x_sb[j], in1=xh_t,
                    op=mybir.AluOpType.subtract)

            pend = None
            for j in range(NB):
                split_band(j)
                xh_t, xl_t = xh_sb[j], xl_sb[j]
                xr_v = xh_t[:].rearrange(
                    "p (i a w b) -> p i a w b", i=8, a=2, w=64, b=2)
                xl_v = xl_t[:].rearrange(
                    "p (i a w b) -> p i a w b", i=8, a=2, w=64, b=2)
                acc_q = conv_band(j, wqh, wql, xr_v, xl_v, None)
                acc_k = conv_band(j, wkh, wkl, xr_v, xl_v, None)

                qc = stage.tile([128, QCHUNK], f32, tag="qchunk")
                kc = stage.tile([128, QCHUNK], f32, tag="kchunk")
                if with_qk_bias:
                    nc.scalar.activation(out=qc, in_=acc_q, func=Ident,
                                         bias=bq_sb[:, 0:1], scale=1.0)
                    nc.scalar.activation(out=kc, in_=acc_k, func=Ident,
                                         bias=bk_sb[:, 0:1], scale=1.0)
                else:
                    nc.scalar.activation(out=qc, in_=acc_q, func=CopyF,
                                         bias=0.0, scale=1.0)
                    nc.scalar.activation(out=kc, in_=acc_k, func=CopyF,
                                         bias=0.0, scale=1.0)
                # transposes + energy one band behind
                if pend is not None:
                    emit_transposes(*pend)
                    emit_energy(pend[0])
                pend = (j, qc, kc)
            emit_transposes(*pend)
            emit_energy(pend[0])

            # keep the PE busy through the softmax serial chain so the HAM
            # clock gate doesn't re-throttle before the output matmuls
            # (results unused; inputs are long since ready)
            for dw in range(28):
                scratch = pacc.tile([128, 256], f32, tag="acc",
                                    name=f"warm{dw}")
                nc.tensor.matmul(
                    scratch, lhsT=wqh[:, 0:128],
                    rhs=xh_sb[0][:, 0:256],
                    start=True, stop=True)

            # softmin over rows: att = exp(rowmin - E) / Z
            mmin = small.tile([128, 1], f32, tag="mmin")
            nc.vector.tensor_reduce(
                out=mmin, in_=E, axis=mybir.AxisListType.X,
                op=mybir.AluOpType.min)
            w_sb = small.tile([128, 128], f32, tag="w")
            zsum = small.tile([128, 1], f32, tag="z")
            nc.scalar.activation(
                out=w_sb, in_=E, func=mybir.ActivationFunctionType.Exp,
                bias=mmin[:, 0:1], scale=-1.0, accum_out=zsum[:, 0:1])
            rz = small.tile([128, 1], f32, tag="rz")
            nc.vector.reciprocal(rz, zsum)
            att = small.tile([128, 128], f32, tag="att")
            nc.vector.tensor_scalar_mul(att, w_sb, rz[:, 0:1])

            attT_p = psm.tile([128, 128], f32, tag="s2")
            nc.tensor.transpose(attT_p, att, ident)
            attT = small.tile([128, 128], f32, tag="attT")
            nc.vector.tensor_copy(attT, attT_p)

            # M^T[c2, c] = sum_d Wv[d, c2] attT[d, c], split into bf16 hi/lo
            MT_p = psm.tile([128, 128], f32, tag="s2")
            nc.tensor.matmul(MT_p, lhsT=wv_sb, rhs=attT, start=True, stop=True)
            Mh = small.tile([128, 128], bf16, tag="Mh")
            nc.vector.tensor_copy(Mh, MT_p)
            Ml = small.tile([128, 128], bf16, tag="Ml")
            nc.vector.tensor_tensor(
                out=Ml, in0=MT_p, in1=Mh, op=mybir.AluOpType.subtract)

            if with_v_bias:
                abv_p = psm.tile([128, 1], f32, tag="s2")
                nc.tensor.matmul(abv_p, lhsT=attT, rhs=bv_sb[:, 0:1],
                                 start=True, stop=True)
                abv = small.tile([128, 1], f32, tag="abv")
                nc.vector.tensor_copy(abv, abv_p)

            # out[c, n] = sum_c2 M[c, c2] x[c2, n] (+ abv[c]) via bf16 split.
            # Stationary-major order within each band: one LDW for Mh across
            # 8 matmuls, one for Ml across 4, with 4 PSUM accumulators in
            # flight.
            out_dma_engines = [nc.sync, nc.gpsimd, nc.scalar]
            for j in range(NB):
                o_band = oout.tile([128, BAND], f32, tag="oband")
                o_ps = [pacc.tile([128, 512], f32, tag="acc",
                                  name=f"ops{j}_{s}")
                        for s in range(4)]
                for s in range(4):
                    nc.tensor.matmul(
                        o_ps[s], lhsT=Mh,
                        rhs=xh_sb[j][:, s * 512:(s + 1) * 512],
                        start=True, stop=False)
                for s in range(4):
                    nc.tensor.matmul(
                        o_ps[s], lhsT=Mh,
                        rhs=xl_sb[j][:, s * 512:(s + 1) * 512],
                        start=False, stop=False)
                for s in range(4):
                    nc.tensor.matmul(
                        o_ps[s], lhsT=Ml,
                        rhs=xh_sb[j][:, s * 512:(s + 1) * 512],
                        start=False, stop=True)
                for s in range(4):
                    dst = o_band[:, s * 512:(s + 1) * 512]
                    if with_v_bias:
                        nc.scalar.activation(
                            out=dst, in_=o_ps[s], func=Ident,
                            bias=abv[:, 0:1], scale=1.0)
                    elif s % 2 == 0:
                        nc.vector.tensor_copy(dst, o_ps[s])
                    else:
                        nc.scalar.activation(out=dst, in_=o_ps[s], func=CopyF,
                                             bias=0.0, scale=1.0)
                pieces = 2 if j == NB - 1 else 1
                psz = BAND // pieces
                for h in range(pieces):
                    off = j * BAND + h * psz
                    out_dma_engines[(j + h) % 3].dma_start(
                        out=out_d[:, off:off + psz],
                        in_=o_band[:, h * psz:(h + 1) * psz])

    nc.compile()
    return nc


def kernel(x, Wq, bq, Wk, bk, Wv, bv):
    from concourse.bass_utils import run_bass_kernel_spmd

    x = np.ascontiguousarray(np.asarray(x, dtype=np.float32))
    Wq = np.asarray(Wq, dtype=np.float32)
    Wk = np.asarray(Wk, dtype=np.float32)
    Wv = np.asarray(Wv, dtype=np.float32)
    bq = np.asarray(bq, dtype=np.float32)
    bk = np.asarray(bk, dtype=np.float32)
    bv = np.asarray(bv, dtype=np.float32)

    with_qk_bias = bool(np.any(bq) or np.any(bk))
    with_v_bias = bool(np.any(bv))

    key = (with_qk_bias, with_v_bias)
    if key not in _CACHE:
        _CACHE[key] = _build_program(with_qk_bias, with_v_bias)
    nc = _CACHE[key]

    # weight layout prep: wT[cin, ab*128 + c] = W[c, cin, a, b];
    # q and k weights packed into one tensor for a single early DMA
    wqT = Wq.transpose(1, 2, 3, 0).reshape(C, 4 * C)
    wkT = Wk.transpose(1, 2, 3, 0).reshape(C, 4 * C)
    wqk = np.ascontiguousarray(np.concatenate([wqT, wkT], axis=1))
    wv = np.ascontiguousarray(Wv.reshape(C, C))

    in_maps = []
    for b in range(B):
        m = {
            "x": np.ascontiguousarray(x[b].reshape(C, HW)),
            "wqk": wqk,
            "wv": wv,
        }
        if with_qk_bias:
            m["bq"] = np.ascontiguousarray(bq.reshape(C, 1))
            m["bk"] = np.ascontiguousarray(bk.reshape(C, 1))
        if with_v_bias:
            m["bv"] = np.ascontiguousarray(bv.reshape(C, 1))
        in_maps.append(m)

    res = run_bass_kernel_spmd(nc, in_maps, list(range(N_CORES)))
    out = np.stack([res.results[i]["out"] for i in range(N_CORES)])
    return out.reshape(B, C, H, W).astype(np.float32)



# revision 3
# speedup vs baseline: 1.0094x; 1.0094x over previous
"""CAM (channel attention) module kernel for Trainium2, 8-core data-parallel.

Reference computation (per sample b):
    q = conv2d(x, Wq, stride2, 2x2) -> [C, 4096]
    k = conv2d(x, Wk, stride2, 2x2) -> [C, 4096]
    v = conv2d(x, Wv, 1x1)          -> [C, 16384]
    E = q @ k^T                      [C, C]
    att = softmax(rowmax(E) - E)   (== softmin over rows)
    out = att @ v -> [C, H, W]

Kernel strategy (one sample per NeuronCore, B=8 over 8 cores):
  - The softmax here is extremely peaked (energy entries span +-200), so
    energy errors are amplified exponentially: q/k need ~16+ mantissa
    bits, which rules out bf16/fp16 single- or 2-pass schemes for the
    convs (2-pass fp16 measures 2.2e-2 rel err vs the 2e-2 gate).
    The cheapest precise scheme is split-bf16: x = xh + xl with
    xh = bf16(x), xl = bf16(x - xh) (~16-bit combined); same for the
    conv weights. conv = Wh@xh + Wh@xl + Wl@xh: 3 full-rate bf16
    passes (bf16 moving operands stream 1 cyc/row vs fp32's 4).
  - Startup: wqk loads first (small, needed by every conv matmul), then
    band 0 in two half-band pieces so the DVE split overlaps the DMA,
    then bands 1-7. Dummy ident matmuls pre-warm the PE clock ramp
    while the input streams in.
  - conv emits pass-major (all xh-matmuls before xl) with q/k
    interleaved, so a band's convs can start before its xl split is
    done; stationary (weight) loads are fully hidden under the streams.
  - conv produces q in [c, n] layout via strided im2col APs from the
    resident xh/xl tiles, then PE-transposes to [n, c] chunks for the
    energy contraction. Energy chunk matmuls interleave into the conv
    stream (one piece behind) so the PE never idles.
  - energy e = q k^T in native fp32 (exact; N=128 makes fp32r slow
    there anyway), accumulated over 32 chunk matmuls in one PSUM bank.
  - softmin via one DVE row-min + one ScalarE exp (bias=rowmin,
    scale=-1) with fused accumulated row-sum. Normalization is
    DEFERRED: the unnormalized w = exp(min - E) feeds the value path
    and the final copies scale by 1/z per partition (activation
    scale-AP / DVE tensor_scalar), shortening the serial chain.
  - out = att @ (Wv x + bv) == (1/z) * (w Wv) @ x (+ scaled bias):
    computes M'^T = Wv^T w^T on PE ([128,128]), casts to bf16 once,
    and runs out = Mh@xh + Mh@xl against the resident split-x tiles
    (2 bf16 passes, one stationary load; dropping the Ml@xh term
    costs ~2e-4 rel err). The phase is store-DMA-paced, so the saved
    third pass mostly shortens the lead-in/tail.
"""

import numpy as np

B, C, H, W = 8, 128, 128, 128
HW = H * W           # 16384
N_CORES = 8
NB = 8               # number of H-bands (16 input rows each)
BAND = HW // NB      # 2048 x columns per band
HALF = BAND // 2     # 1024
QN = (H // 2) * (W // 2)  # 4096 conv output positions
QCHUNK = QN // NB    # 512 conv outputs per band

OUT_PASSES = 2       # bf16 passes in the output matmul (1..3)
N_PREWARM = 30       # PE ramp filler during input DMA
N_SOFTWARM = 14      # PE filler across the softmax serial chain

_CACHE = {}


def _build_program(with_qk_bias: bool, with_v_bias: bool):
    import concourse.tile as tile
    from concourse import bacc, mybir
    from concourse.masks import make_identity

    f32 = mybir.dt.float32
    bf16 = mybir.dt.bfloat16
    Ident = mybir.ActivationFunctionType.Identity
    CopyF = mybir.ActivationFunctionType.Copy
    nc = bacc.Bacc(
        "TRN2", target_bir_lowering=False, debug=False, num_devices=N_CORES)

    x_d = nc.declare_dram_parameter("x", [C, HW], f32, isOutput=False)
    wqk_d = nc.declare_dram_parameter("wqk", [C, 8 * C], f32, isOutput=False)
    wv_d = nc.declare_dram_parameter("wv", [C, C], f32, isOutput=False)
    if with_qk_bias:
        bq_d = nc.declare_dram_parameter("bq", [C, 1], f32, isOutput=False)
        bk_d = nc.declare_dram_parameter("bk", [C, 1], f32, isOutput=False)
    if with_v_bias:
        bv_d = nc.declare_dram_parameter("bv", [C, 1], f32, isOutput=False)
    out_d = nc.declare_dram_parameter("out", [C, HW], f32, isOutput=True)

    with tile.TileContext(nc) as tc:
        with (
            tc.tile_pool(name="const", bufs=1) as const,
            tc.tile_pool(name="xstage", bufs=3) as xstage,
            tc.tile_pool(name="xrp", bufs=1) as xrp,
            tc.tile_pool(name="qkT", bufs=1) as qkT,
            tc.tile_pool(name="stage", bufs=3) as stage,
            tc.tile_pool(name="oout", bufs=3) as oout,
            tc.tile_pool(name="small", bufs=2) as small,
            tc.tile_pool(name="pacc", bufs=4, space="PSUM") as pacc,
            tc.tile_pool(name="ptp", bufs=2, space="PSUM") as ptp,
            tc.tile_pool(name="psm", bufs=1, space="PSUM") as psm,
        ):
            ident = const.tile([128, 128], f32, tag="ident")
            make_identity(nc, ident)

            # Startup: per-core DMA bandwidth is ONE shared resource, so
            # serialize ALL input loads on the sync queue in consumption
            # order. wqk goes first (every conv matmul needs it), then
            # band 0 as two half-bands so the DVE split pipeline starts
            # sooner, then the remaining bands.
            wqk_sb = const.tile([C, 8 * C], f32, tag="wqk")
            nc.sync.dma_start(out=wqk_sb, in_=wqk_d[:, :])
            wqT_sb = wqk_sb[:, 0:4 * C]
            wkT_sb = wqk_sb[:, 4 * C:8 * C]
            if with_qk_bias:
                bq_sb = const.tile([C, 1], f32, tag="bq")
                nc.sync.dma_start(out=bq_sb, in_=bq_d[:, :])
                bk_sb = const.tile([C, 1], f32, tag="bk")
                nc.sync.dma_start(out=bk_sb, in_=bk_d[:, :])
            x_sb = []
            x0 = xstage.tile([C, BAND], f32, tag="x", name="x0")
            nc.sync.dma_start(out=x0[:, 0:HALF], in_=x_d[:, 0:HALF])
            nc.sync.dma_start(out=x0[:, HALF:BAND], in_=x_d[:, HALF:BAND])
            x_sb.append(x0)
            for j in range(1, NB):
                t = xstage.tile([C, BAND], f32, tag="x", name=f"x{j}")
                nc.sync.dma_start(out=t, in_=x_d[:, j * BAND:(j + 1) * BAND])
                x_sb.append(t)
            wv_sb = const.tile([C, C], f32, tag="wv")
            nc.sync.dma_start(out=wv_sb, in_=wv_d[:, :])
            if with_v_bias:
                bv_sb = const.tile([C, 1], f32, tag="bv")
                nc.sync.dma_start(out=bv_sb, in_=bv_d[:, :])

            # PE clock pre-warm: the tensor engine ramps to full clock
            # only after ~3us of continuous work, and the first real
            # matmul can't start until band 0 + weights have landed
            # (~12us in). Fill the wait with throwaway ident matmuls so
            # the real conv stream runs at full clock from its first
            # instruction.
            for dw in range(N_PREWARM):
                ws = pacc.tile([128, 128], f32, tag="acc", name=f"pw{dw}")
                nc.tensor.matmul(ws, lhsT=ident, rhs=ident,
                                 start=True, stop=True)

            # split the conv weights into bf16 hi/lo; keep DVE free for
            # the x band-0 splits (hi cast on ScalarE, lo sub on GpSimd)
            def split_w(w_f32, name):
                hi = const.tile([C, 4 * C], bf16, tag=f"{name}hi")
                nc.scalar.activation(out=hi, in_=w_f32, func=CopyF,
                                     bias=0.0, scale=1.0)
                lo = const.tile([C, 4 * C], bf16, tag=f"{name}lo")
                nc.gpsimd.tensor_tensor(
                    out=lo, in0=w_f32, in1=hi,
                    op=mybir.AluOpType.subtract)
                return hi, lo

            wqh, wql = split_w(wqT_sb, "wq")
            wkh, wkl = split_w(wkT_sb, "wk")

            xh_sb = [xrp.tile([C, BAND], bf16, tag=f"xh{j}", name=f"xh{j}")
                     for j in range(NB)]
            xl_sb = [xrp.tile([C, BAND], bf16, tag=f"xl{j}", name=f"xl{j}")
                     for j in range(NB)]

            def split_cols(j, lo, hi):
                nc.vector.tensor_copy(xh_sb[j][:, lo:hi], x_sb[j][:, lo:hi])
                nc.vector.tensor_tensor(
                    out=xl_sb[j][:, lo:hi], in0=x_sb[j][:, lo:hi],
                    in1=xh_sb[j][:, lo:hi], op=mybir.AluOpType.subtract)

            qT = [qkT.tile([128, QCHUNK], f32, tag=f"qT{j}", name=f"qT{j}")
                  for j in range(NB)]
            kT = [qkT.tile([128, QCHUNK], f32, tag=f"kT{j}", name=f"kT{j}")
                  for j in range(NB)]

            # energy accumulator lives across the whole conv phase:
            # energy chunk matmuls are interleaved into the conv stream
            # so the PE never sits in a low-duty phase.
            E = psm.tile([128, 128], f32, tag="E")
            e_idx = [0]

            def conv_piece(j, lo, ncols):
                """q and k convs for x columns [lo, lo+ncols) of band j,
                interleaved pass-major: all xh matmuls first so the
                piece's xl split can finish in their shadow. Returns the
                two PSUM accumulators ([128, ncols//4] each)."""
                i_cnt = ncols // 256
                nout = ncols // 4
                xh_v = xh_sb[j][:, lo:lo + ncols].rearrange(
                    "p (i a w b) -> p i a w b", i=i_cnt, a=2, w=64, b=2)
                xl_v = xl_sb[j][:, lo:lo + ncols].rearrange(
                    "p (i a w b) -> p i a w b", i=i_cnt, a=2, w=64, b=2)
                acc_q = pacc.tile([128, 512], f32, tag="acc",
                                  name=f"aq{j}_{lo}")
                acc_k = pacc.tile([128, 512], f32, tag="acc",
                                  name=f"ak{j}_{lo}")
                n_mm = 0
                for wq_s, wk_s, xv in ((wqh, wkh, xh_v), (wqh, wkh, xl_v),
                                       (wql, wkl, xh_v)):
                    for ab in range(4):
                        a, bb = ab // 2, ab % 2
                        rhs = xv[:, :, a, :, bb]
                        nc.tensor.matmul(
                            acc_q[:, 0:nout],
                            lhsT=wq_s[:, ab * C:(ab + 1) * C], rhs=rhs,
                            start=(n_mm == 0), stop=(n_mm == 11))
                        nc.tensor.matmul(
                            acc_k[:, 0:nout],
                            lhsT=wk_s[:, ab * C:(ab + 1) * C], rhs=rhs,
                            start=(n_mm == 0), stop=(n_mm == 11))
                        n_mm += 1
                return acc_q, acc_k

            def emit_tp_energy(j, qoff, nout, qc, kc):
                for T_out, src in ((qT[j], qc), (kT[j], kc)):
                    tp = ptp.tile([128, 512], f32, tag="tp")
                    for t in range(nout // 128):
                        nc.tensor.transpose(
                            tp[:, t * 128:(t + 1) * 128],
                            src[:, t * 128:(t + 1) * 128], ident)
                    nc.scalar.activation(
                        out=T_out[:, qoff:qoff + nout], in_=tp[:, 0:nout],
                        func=CopyF, bias=0.0, scale=1.0)
                for t in range(nout // 128):
                    o = qoff + t * 128
                    nc.tensor.matmul(
                        E,
                        lhsT=qT[j][:, o:o + 128],
                        rhs=kT[j][:, o:o + 128],
                        start=(e_idx[0] == 0), stop=(e_idx[0] == NB * 4 - 1))
                    e_idx[0] += 1

            pieces = [(0, 0, HALF), (0, HALF, HALF)]
            pieces += [(j, 0, BAND) for j in range(1, NB)]

            pend = None
            for (j, lo, ncols) in pieces:
                split_cols(j, lo, lo + ncols)
                acc_q, acc_k = conv_piece(j, lo, ncols)
                nout = ncols // 4
                qc = stage.tile([128, 512], f32, tag="qchunk",
                                name=f"qc{j}_{lo}")
                kc = stage.tile([128, 512], f32, tag="kchunk",
                                name=f"kc{j}_{lo}")
                if with_qk_bias:
                    nc.scalar.activation(out=qc[:, 0:nout],
                                         in_=acc_q[:, 0:nout], func=Ident,
                                         bias=bq_sb[:, 0:1], scale=1.0)
                    nc.scalar.activation(out=kc[:, 0:nout],
                                         in_=acc_k[:, 0:nout], func=Ident,
                                         bias=bk_sb[:, 0:1], scale=1.0)
                else:
                    nc.scalar.activation(out=qc[:, 0:nout],
                                         in_=acc_q[:, 0:nout], func=CopyF,
                                         bias=0.0, scale=1.0)
                    nc.scalar.activation(out=kc[:, 0:nout],
                                         in_=acc_k[:, 0:nout], func=CopyF,
                                         bias=0.0, scale=1.0)
                # transposes + energy one piece behind
                if pend is not None:
                    emit_tp_energy(*pend)
                pend = (j, lo // 4, nout, qc, kc)
            emit_tp_energy(*pend)

            # keep the PE busy through the softmax serial chain so the
            # clock gate doesn't re-throttle before the output matmuls
            # (results unused; inputs are long since ready)
            for dw in range(N_SOFTWARM):
                scratch = pacc.tile([128, 256], f32, tag="acc",
                                    name=f"warm{dw}")
                nc.tensor.matmul(
                    scratch, lhsT=wqh[:, 0:128],
                    rhs=xh_sb[0][:, 0:256],
                    start=True, stop=True)

            # softmin over rows, normalization deferred: w = exp(min-E),
            # z = rowsum(w); out picks up the 1/z scale in its final
            # copies, so the recip runs off the critical chain.
            mmin = small.tile([128, 1], f32, tag="mmin")
            nc.vector.tensor_reduce(
                out=mmin, in_=E, axis=mybir.AxisListType.X,
                op=mybir.AluOpType.min)
            w_sb = small.tile([128, 128], f32, tag="w")
            zsum = small.tile([128, 1], f32, tag="z")
            nc.scalar.activation(
                out=w_sb, in_=E, func=mybir.ActivationFunctionType.Exp,
                bias=mmin[:, 0:1], scale=-1.0, accum_out=zsum[:, 0:1])

            wT_p = psm.tile([128, 128], f32, tag="s2")
            nc.tensor.transpose(wT_p, w_sb, ident)
            rz = small.tile([128, 1], f32, tag="rz")
            nc.vector.reciprocal(rz, zsum)
            wT = small.tile([128, 128], f32, tag="wT")
            nc.vector.tensor_copy(wT, wT_p)

            # M'^T[c2, c] = sum_d Wv[d, c2] wT[d, c] (unnormalized), cast
            # to bf16 once; the dropped lo-part costs ~2e-4 rel err.
            MT_p = psm.tile([128, 128], f32, tag="s2")
            nc.tensor.matmul(MT_p, lhsT=wv_sb, rhs=wT, start=True, stop=True)
            Mh = small.tile([128, 128], bf16, tag="Mh")
            nc.vector.tensor_copy(Mh, MT_p)
            if OUT_PASSES >= 3:
                Ml = small.tile([128, 128], bf16, tag="Ml")
                nc.vector.tensor_tensor(
                    out=Ml, in0=MT_p, in1=Mh, op=mybir.AluOpType.subtract)

            if with_v_bias:
                # abv' = w @ bv (unnormalized); final copies compute
                # rz*in + rz*abv', so pre-scale the bias on DVE.
                abv_p = psm.tile([128, 1], f32, tag="s2")
                nc.tensor.matmul(abv_p, lhsT=wT, rhs=bv_sb[:, 0:1],
                                 start=True, stop=True)
                abv = small.tile([128, 1], f32, tag="abv")
                nc.vector.tensor_copy(abv, abv_p)
                abvz = small.tile([128, 1], f32, tag="abvz")
                nc.vector.tensor_tensor(
                    out=abvz, in0=abv, in1=rz, op=mybir.AluOpType.mult)

            # out[c, n] = rz[c] * sum_c2 M'[c, c2] x[c2, n] (+ bias) via
            # bf16 passes; one stationary load of Mh per band covers all
            # its matmuls, 4 PSUM accumulators in flight. The phase is
            # store-DMA-paced, so stores start as early as possible and
            # the last band is split 4 ways across queues to shrink the
            # tail.
            out_dma_engines = [nc.sync, nc.gpsimd, nc.scalar]
            for j in range(NB):
                o_band = oout.tile([128, BAND], f32, tag="oband")
                o_ps = [pacc.tile([128, 512], f32, tag="acc",
                                  name=f"ops{j}_{s}")
                        for s in range(4)]
                for s in range(4):
                    nc.tensor.matmul(
                        o_ps[s], lhsT=Mh,
                        rhs=xh_sb[j][:, s * 512:(s + 1) * 512],
                        start=True, stop=(OUT_PASSES == 1))
                if OUT_PASSES >= 2:
                    for s in range(4):
                        nc.tensor.matmul(
                            o_ps[s], lhsT=Mh,
                            rhs=xl_sb[j][:, s * 512:(s + 1) * 512],
                            start=False, stop=(OUT_PASSES == 2))
                if OUT_PASSES >= 3:
                    for s in range(4):
                        nc.tensor.matmul(
                            o_ps[s], lhsT=Ml,
                            rhs=xh_sb[j][:, s * 512:(s + 1) * 512],
                            start=False, stop=True)
                for s in range(4):
                    dst = o_band[:, s * 512:(s + 1) * 512]
                    if with_v_bias:
                        nc.scalar.activation(
                            out=dst, in_=o_ps[s], func=Ident,
                            bias=abvz[:, 0:1], scale=rz[:, 0:1])
                    elif s % 2 == 0:
                        nc.vector.tensor_scalar_mul(dst, o_ps[s], rz[:, 0:1])
                    else:
                        nc.scalar.activation(out=dst, in_=o_ps[s], func=CopyF,
                                             bias=0.0, scale=rz[:, 0:1])
                n_pieces = 4 if j == NB - 1 else 1
                psz = BAND // n_pieces
                for h in range(n_pieces):
                    off = j * BAND + h * psz
                    out_dma_engines[(j + h) % 3].dma_start(
                        out=out_d[:, off:off + psz],
                        in_=o_band[:, h * psz:(h + 1) * psz])

    nc.compile()
    return nc


def kernel(x, Wq, bq, Wk, bk, Wv, bv):
    from concourse.bass_utils import run_bass_kernel_spmd

    x = np.ascontiguousarray(np.asarray(x, dtype=np.float32))
    Wq = np.asarray(Wq, dtype=np.float32)
    Wk = np.asarray(Wk, dtype=np.float32)
    Wv = np.asarray(Wv, dtype=np.float32)
    bq = np.asarray(bq, dtype=np.float32)
    bk = np.asarray(bk, dtype=np.float32)
    bv = np.asarray(bv, dtype=np.float32)

    with_qk_bias = bool(np.any(bq) or np.any(bk))
    with_v_bias = bool(np.any(bv))

    key = (with_qk_bias, with_v_bias)
    if key not in _CACHE:
        _CACHE[key] = _build_program(with_qk_bias, with_v_bias)
    nc = _CACHE[key]

    # weight layout prep: wT[cin, ab*128 + c] = W[c, cin, a, b];
    # q and k weights packed into one tensor for a single early DMA
    wqT = Wq.transpose(1, 2, 3, 0).reshape(C, 4 * C)
    wkT = Wk.transpose(1, 2, 3, 0).reshape(C, 4 * C)
    wqk = np.ascontiguousarray(np.concatenate([wqT, wkT], axis=1))
    wv = np.ascontiguousarray(Wv.reshape(C, C))

    in_maps = []
    for b in range(B):
        m = {
            "x": np.ascontiguousarray(x[b].reshape(C, HW)),
            "wqk": wqk,
            "wv": wv,
        }
        if with_qk_bias:
            m["bq"] = np.ascontiguousarray(bq.reshape(C, 1))
            m["bk"] = np.ascontiguousarray(bk.reshape(C, 1))
        if with_v_bias:
            m["bv"] = np.ascontiguousarray(bv.reshape(C, 1))
        in_maps.append(m)

    res = run_bass_kernel_spmd(nc, in_maps, list(range(N_CORES)))
    out = np.stack([res.results[i]["out"] for i in range(N_CORES)])
    return out.reshape(B, C, H, W).astype(np.float32)


# revision 13
# speedup vs baseline: 1.0359x; 1.0263x over previous
"""CAM (channel attention) module kernel for Trainium2, 8-core data-parallel.

Reference computation (per sample b):
    q = conv2d(x, Wq, stride2, 2x2) -> [C, 4096]
    k = conv2d(x, Wk, stride2, 2x2) -> [C, 4096]
    v = conv2d(x, Wv, 1x1)          -> [C, 16384]
    E = q @ k^T                      [C, C]
    att = softmax(rowmax(E) - E)   (== softmin over rows)
    out = att @ v -> [C, H, W]

Kernel strategy (one sample per NeuronCore, B=8 over 8 cores):
  - The softmax here is extremely peaked (energy entries span +-200), so
    energy errors are amplified exponentially: q/k need ~16+ mantissa
    bits, which rules out bf16/fp16 single- or 2-pass schemes for the
    convs (2-pass fp16 measures 2.2e-2 rel err vs the 2e-2 gate).
    The cheapest precise scheme is split-bf16: x = xh + xl with
    xh = bf16(x), xl = bf16(x - xh) (~16-bit combined); same for the
    conv weights. conv = Wh@xh + Wh@xl + Wl@xh: 3 full-rate bf16
    passes (bf16 moving operands stream 1 cyc/row vs fp32's 4).
    (A transposed x-stationary formulation would skip the q/k
    transposes, but the PE stationary operand must be a single-stride
    AP and the im2col tap views are not — verified to fail BIR.)
  - Startup: wqk loads first (small, needed by every conv matmul, with
    per-tap bf16 splits so tap 0 is ready ~0.5us after it lands), then
    band 0 in two half-band pieces so the DVE split overlaps the DMA,
    then bands 1-7. Dummy ident matmuls pre-warm the PE clock ramp
    while the input streams in.
  - conv emits pass-major (all xh-matmuls before xl) with q/k
    interleaved, so a band's convs can start before its xl split is
    done; stationary (weight) loads are fully hidden under the streams.
  - conv produces q in [c, n] layout via strided im2col APs from the
    resident xh/xl tiles, then PE-transposes to [n, c] chunks for the
    energy contraction. Transposes + energy chunk matmuls interleave
    into the conv stream (one piece behind) so the PE never idles.
  - energy e = q k^T in native fp32 (exact), accumulated over 32 chunk
    matmuls in one PSUM bank.
  - softmin via one DVE row-min + one ScalarE exp (bias=rowmin,
    scale=-1) with fused accumulated row-sum. Normalization is
    DEFERRED: the unnormalized w = exp(min - E) feeds the value path
    and the final copies scale by 1/z per partition, keeping the
    reciprocal off the serial chain.
  - out = att @ (Wv x + bv) == (1/z) * (w Wv) @ x (+ scaled bias):
    computes M'^T = Wv^T w^T on PE ([128,128]), casts to bf16 once,
    and runs out = Mh@xh against the resident split-x tiles (1 bf16
    pass, one stationary load; the dropped lo terms cost ~2.2e-3 rel
    err total vs the 2e-2 gate). The phase is store-DMA-paced: stores
    start right after each band's first PSUM copy, the first band is
    split 2x and the last 4x across queues to shrink lead-in/tail.
"""

import numpy as np

B, C, H, W = 8, 128, 128, 128
HW = H * W           # 16384
N_CORES = 8
NB = 8               # number of H-bands (16 input rows each)
BAND = HW // NB      # 2048 x columns per band
HALF = BAND // 2     # 1024
QN = (H // 2) * (W // 2)  # 4096 conv output positions
QCHUNK = QN // NB    # 512 conv outputs per band

OUT_PASSES = 1       # bf16 passes in the output matmul (1..2)
N_PREWARM = 16       # PE ramp filler during input DMA
N_SOFTWARM = 14      # PE filler across the softmax serial chain

_CACHE = {}


def _build_program(with_qk_bias: bool, with_v_bias: bool):
    import concourse.tile as tile
    from concourse import bacc, mybir
    from concourse.masks import make_identity

    f32 = mybir.dt.float32
    bf16 = mybir.dt.bfloat16
    Ident = mybir.ActivationFunctionType.Identity
    CopyF = mybir.ActivationFunctionType.Copy
    nc = bacc.Bacc(
        "TRN2", target_bir_lowering=False, debug=False, num_devices=N_CORES)

    x_d = nc.declare_dram_parameter("x", [C, HW], f32, isOutput=False)
    wqk_d = nc.declare_dram_parameter("wqk", [C, 8 * C], f32, isOutput=False)
    wv_d = nc.declare_dram_parameter("wv", [C, C], f32, isOutput=False)
    if with_qk_bias:
        bq_d = nc.declare_dram_parameter("bq", [C, 1], f32, isOutput=False)
        bk_d = nc.declare_dram_parameter("bk", [C, 1], f32, isOutput=False)
    if with_v_bias:
        bv_d = nc.declare_dram_parameter("bv", [C, 1], f32, isOutput=False)
    out_d = nc.declare_dram_parameter("out", [C, HW], f32, isOutput=True)

    with tile.TileContext(nc) as tc:
        with (
            tc.tile_pool(name="const", bufs=1) as const,
            tc.tile_pool(name="xstage", bufs=3) as xstage,
            tc.tile_pool(name="xrp", bufs=1) as xrp,
            tc.tile_pool(name="qkT", bufs=1) as qkT,
            tc.tile_pool(name="stage", bufs=3) as stage,
            tc.tile_pool(name="oout", bufs=3) as oout,
            tc.tile_pool(name="small", bufs=2) as small,
            tc.tile_pool(name="pacc", bufs=5, space="PSUM") as pacc,
            tc.tile_pool(name="ptp", bufs=2, space="PSUM") as ptp,
            tc.tile_pool(name="psm", bufs=1, space="PSUM") as psm,
        ):
            ident = const.tile([128, 128], f32, tag="ident")
            make_identity(nc, ident)

            # Startup: per-core DMA bandwidth is ONE shared resource, so
            # serialize ALL input loads on the sync queue in consumption
            # order: wqk (every conv matmul needs it), band 0 as two
            # half-band pieces (the DVE split pipeline starts after the
            # first 0.5MiB), then the remaining bands.
            wqk_sb = const.tile([C, 8 * C], f32, tag="wqk")
            nc.sync.dma_start(out=wqk_sb, in_=wqk_d[:, :])
            wqT_sb = wqk_sb[:, 0:4 * C]
            wkT_sb = wqk_sb[:, 4 * C:8 * C]
            if with_qk_bias:
                bq_sb = const.tile([C, 1], f32, tag="bq")
                nc.sync.dma_start(out=bq_sb, in_=bq_d[:, :])
                bk_sb = const.tile([C, 1], f32, tag="bk")
                nc.sync.dma_start(out=bk_sb, in_=bk_d[:, :])
            x_sb = []
            x0 = xstage.tile([C, BAND], f32, tag="x", name="x0")
            nc.sync.dma_start(out=x0[:, 0:HALF], in_=x_d[:, 0:HALF])
            nc.sync.dma_start(out=x0[:, HALF:BAND], in_=x_d[:, HALF:BAND])
            x_sb.append(x0)
            for j in range(1, NB):
                t = xstage.tile([C, BAND], f32, tag="x", name=f"x{j}")
                nc.sync.dma_start(out=t, in_=x_d[:, j * BAND:(j + 1) * BAND])
                x_sb.append(t)
            wv_sb = const.tile([C, C], f32, tag="wv")
            nc.sync.dma_start(out=wv_sb, in_=wv_d[:, :])
            if with_v_bias:
                bv_sb = const.tile([C, 1], f32, tag="bv")
                nc.sync.dma_start(out=bv_sb, in_=bv_d[:, :])

            # PE clock pre-warm: the tensor engine ramps to full clock
            # only after ~3us of continuous work, and the first real
            # matmul can't start until weights + band 0 have landed.
            # Fill the wait with throwaway ident matmuls.
            for dw in range(N_PREWARM):
                ws = pacc.tile([128, 128], f32, tag="acc", name=f"pw{dw}")
                nc.tensor.matmul(ws, lhsT=ident, rhs=ident,
                                 start=True, stop=True)

            # split the conv weights into bf16 hi/lo PER TAP, tap-major
            # q-then-k, so the first conv matmul's weights are ready
            # right after wqk lands (hi cast on ScalarE, lo sub on
            # GpSimd; DVE stays free for the x splits)
            wqh = const.tile([C, 4 * C], bf16, tag="wqh")
            wql = const.tile([C, 4 * C], bf16, tag="wql")
            wkh = const.tile([C, 4 * C], bf16, tag="wkh")
            wkl = const.tile([C, 4 * C], bf16, tag="wkl")
            for ab in range(4):
                s = slice(ab * C, (ab + 1) * C)
                for (hi, lo, src) in ((wqh, wql, wqT_sb), (wkh, wkl, wkT_sb)):
                    nc.scalar.activation(out=hi[:, s], in_=src[:, s],
                                         func=CopyF, bias=0.0, scale=1.0)
                    nc.gpsimd.tensor_tensor(
                        out=lo[:, s], in0=src[:, s], in1=hi[:, s],
                        op=mybir.AluOpType.subtract)

            xh_sb = [xrp.tile([C, BAND], bf16, tag=f"xh{j}", name=f"xh{j}")
                     for j in range(NB)]
            xl_sb = [xrp.tile([C, BAND], bf16, tag=f"xl{j}", name=f"xl{j}")
                     for j in range(NB)]

            def split_cols(j, lo, hi):
                nc.vector.tensor_copy(xh_sb[j][:, lo:hi], x_sb[j][:, lo:hi])
                nc.vector.tensor_tensor(
                    out=xl_sb[j][:, lo:hi], in0=x_sb[j][:, lo:hi],
                    in1=xh_sb[j][:, lo:hi], op=mybir.AluOpType.subtract)

            qT = [qkT.tile([128, QCHUNK], f32, tag=f"qT{j}", name=f"qT{j}")
                  for j in range(NB)]
            kT = [qkT.tile([128, QCHUNK], f32, tag=f"kT{j}", name=f"kT{j}")
                  for j in range(NB)]

            # energy accumulator lives across the whole conv phase:
            # energy chunk matmuls are interleaved into the conv stream
            # so the PE never sits in a low-duty phase.
            E = psm.tile([128, 128], f32, tag="E")
            e_idx = [0]

            def conv_piece(j, lo, ncols):
                """q and k convs for x columns [lo, lo+ncols) of band j,
                interleaved pass-major: all xh matmuls first so the
                piece's xl split can finish in their shadow. Returns the
                two PSUM accumulators ([128, ncols//4] each)."""
                i_cnt = ncols // 256
                nout = ncols // 4
                xh_v = xh_sb[j][:, lo:lo + ncols].rearrange(
                    "p (i a w b) -> p i a w b", i=i_cnt, a=2, w=64, b=2)
                xl_v = xl_sb[j][:, lo:lo + ncols].rearrange(
                    "p (i a w b) -> p i a w b", i=i_cnt, a=2, w=64, b=2)
                acc_q = pacc.tile([128, 512], f32, tag="acc",
                                  name=f"aq{j}_{lo}")
                acc_k = pacc.tile([128, 512], f32, tag="acc",
                                  name=f"ak{j}_{lo}")
                n_mm = 0
                for wq_s, wk_s, xv in ((wqh, wkh, xh_v), (wqh, wkh, xl_v),
                                       (wql, wkl, xh_v)):
                    for ab in range(4):
                        a, bb = ab // 2, ab % 2
                        rhs = xv[:, :, a, :, bb]
                        nc.tensor.matmul(
                            acc_q[:, 0:nout],
                            lhsT=wq_s[:, ab * C:(ab + 1) * C], rhs=rhs,
                            start=(n_mm == 0), stop=(n_mm == 11))
                        nc.tensor.matmul(
                            acc_k[:, 0:nout],
                            lhsT=wk_s[:, ab * C:(ab + 1) * C], rhs=rhs,
                            start=(n_mm == 0), stop=(n_mm == 11))
                        n_mm += 1
                return acc_q, acc_k

            def emit_tp_energy(j, qoff, nout, qc, kc):
                for T_out, src in ((qT[j], qc), (kT[j], kc)):
                    tp = ptp.tile([128, 512], f32, tag="tp")
                    for t in range(nout // 128):
                        nc.tensor.transpose(
                            tp[:, t * 128:(t + 1) * 128],
                            src[:, t * 128:(t + 1) * 128], ident)
                    nc.scalar.activation(
                        out=T_out[:, qoff:qoff + nout], in_=tp[:, 0:nout],
                        func=CopyF, bias=0.0, scale=1.0)
                for t in range(nout // 128):
                    o = qoff + t * 128
                    nc.tensor.matmul(
                        E,
                        lhsT=qT[j][:, o:o + 128],
                        rhs=kT[j][:, o:o + 128],
                        start=(e_idx[0] == 0), stop=(e_idx[0] == NB * 4 - 1))
                    e_idx[0] += 1

            pieces = [(0, 0, HALF), (0, HALF, HALF)]
            pieces += [(j, 0, BAND) for j in range(1, NB)]

            pend = None
            for (j, lo, ncols) in pieces:
                split_cols(j, lo, lo + ncols)
                acc_q, acc_k = conv_piece(j, lo, ncols)
                nout = ncols // 4
                qc = stage.tile([128, 512], f32, tag="qchunk",
                                name=f"qc{j}_{lo}")
                kc = stage.tile([128, 512], f32, tag="kchunk",
                                name=f"kc{j}_{lo}")
                if with_qk_bias:
                    nc.scalar.activation(out=qc[:, 0:nout],
                                         in_=acc_q[:, 0:nout], func=Ident,
                                         bias=bq_sb[:, 0:1], scale=1.0)
                    nc.scalar.activation(out=kc[:, 0:nout],
                                         in_=acc_k[:, 0:nout], func=Ident,
                                         bias=bk_sb[:, 0:1], scale=1.0)
                else:
                    nc.scalar.activation(out=qc[:, 0:nout],
                                         in_=acc_q[:, 0:nout], func=CopyF,
                                         bias=0.0, scale=1.0)
                    nc.scalar.activation(out=kc[:, 0:nout],
                                         in_=acc_k[:, 0:nout], func=CopyF,
                                         bias=0.0, scale=1.0)
                # transposes + energy one piece behind
                if pend is not None:
                    emit_tp_energy(*pend)
                pend = (j, lo // 4, nout, qc, kc)
            emit_tp_energy(*pend)

            # keep the PE busy through the softmax serial chain so the
            # clock gate doesn't re-throttle before the output matmuls
            # (results unused; inputs are long since ready)
            for dw in range(N_SOFTWARM):
                scratch = pacc.tile([128, 256], f32, tag="acc",
                                    name=f"warm{dw}")
                nc.tensor.matmul(
                    scratch, lhsT=wqh[:, 0:128],
                    rhs=xh_sb[0][:, 0:256],
                    start=True, stop=True)

            # softmin over rows, normalization deferred: w = exp(min-E),
            # z = rowsum(w); out picks up the 1/z scale in its final
            # copies, so the recip runs off the critical chain.
            mmin = small.tile([128, 1], f32, tag="mmin")
            nc.vector.tensor_reduce(
                out=mmin, in_=E, axis=mybir.AxisListType.X,
                op=mybir.AluOpType.min)
            w_sb = small.tile([128, 128], f32, tag="w")
            zsum = small.tile([128, 1], f32, tag="z")
            nc.scalar.activation(
                out=w_sb, in_=E, func=mybir.ActivationFunctionType.Exp,
                bias=mmin[:, 0:1], scale=-1.0, accum_out=zsum[:, 0:1])

            wT_p = psm.tile([128, 128], f32, tag="E")
            nc.tensor.transpose(wT_p, w_sb, ident)
            rz = small.tile([128, 1], f32, tag="rz")
            nc.vector.reciprocal(rz, zsum)
            wT = small.tile([128, 128], f32, tag="wT")
            nc.vector.tensor_copy(wT, wT_p)

            # M'^T[c2, c] = sum_d Wv[d, c2] wT[d, c] (unnormalized), cast
            # to bf16 once; the dropped lo terms cost ~2.2e-3 rel err.
            MT_p = psm.tile([128, 128], f32, tag="E")
            nc.tensor.matmul(MT_p, lhsT=wv_sb, rhs=wT, start=True, stop=True)
            Mh = small.tile([128, 128], bf16, tag="Mh")
            nc.vector.tensor_copy(Mh, MT_p)

            if with_v_bias:
                # abv' = w @ bv (unnormalized); final copies compute
                # rz*in + rz*abv', so pre-scale the bias on DVE.
                abv_p = psm.tile([128, 1], f32, tag="E")
                nc.tensor.matmul(abv_p, lhsT=wT, rhs=bv_sb[:, 0:1],
                                 start=True, stop=True)
                abv = small.tile([128, 1], f32, tag="abv")
                nc.vector.tensor_copy(abv, abv_p)
                abvz = small.tile([128, 1], f32, tag="abvz")
                nc.vector.tensor_tensor(
                    out=abvz, in0=abv, in1=rz, op=mybir.AluOpType.mult)

            # out[c, n] = rz[c] * sum_c2 M'[c, c2] x[c2, n] (+ bias) via
            # bf16; one stationary load of Mh covers everything, PSUM
            # accumulators rotate through the pool. The phase is
            # store-DMA-paced, so stores start as early as possible: the
            # first band is split 2x and the last 4x across the three
            # store queues to shrink lead-in/tail.
            out_dma_engines = [nc.sync, nc.gpsimd, nc.scalar]
            for j in range(NB):
                o_band = oout.tile([128, BAND], f32, tag="oband")
                o_ps = [pacc.tile([128, 512], f32, tag="acc",
                                  name=f"ops{j}_{s}")
                        for s in range(4)]
                for s in range(4):
                    nc.tensor.matmul(
                        o_ps[s], lhsT=Mh,
                        rhs=xh_sb[j][:, s * 512:(s + 1) * 512],
                        start=True, stop=(OUT_PASSES == 1))
                if OUT_PASSES >= 2:
                    for s in range(4):
                        nc.tensor.matmul(
                            o_ps[s], lhsT=Mh,
                            rhs=xl_sb[j][:, s * 512:(s + 1) * 512],
                            start=False, stop=True)
                for s in range(4):
                    dst = o_band[:, s * 512:(s + 1) * 512]
                    if with_v_bias:
                        nc.scalar.activation(
                            out=dst, in_=o_ps[s], func=Ident,
                            bias=abvz[:, 0:1], scale=rz[:, 0:1])
                    elif s % 2 == 0:
                        nc.vector.tensor_scalar_mul(dst, o_ps[s], rz[:, 0:1])
                    else:
                        nc.scalar.activation(out=dst, in_=o_ps[s], func=CopyF,
                                             bias=0.0, scale=rz[:, 0:1])
                n_pieces = 4 if j == NB - 1 else (2 if j == 0 else 1)
                psz = BAND // n_pieces
                for h in range(n_pieces):
                    off = j * BAND + h * psz
                    out_dma_engines[(j + h) % 3].dma_start(
                        out=out_d[:, off:off + psz],
                        in_=o_band[:, h * psz:(h + 1) * psz])

    nc.compile()
    return nc


def kernel(x, Wq, bq, Wk, bk, Wv, bv):
    from concourse.bass_utils import run_bass_kernel_spmd

    x = np.ascontiguousarray(np.asarray(x, dtype=np.float32))
    Wq = np.asarray(Wq, dtype=np.float32)
    Wk = np.asarray(Wk, dtype=np.float32)
    Wv = np.asarray(Wv, dtype=np.float32)
    bq = np.asarray(bq, dtype=np.float32)
    bk = np.asarray(bk, dtype=np.float32)
    bv = np.asarray(bv, dtype=np.float32)

    with_qk_bias = bool(np.any(bq) or np.any(bk))
    with_v_bias = bool(np.any(bv))

    key = (with_qk_bias, with_v_bias)
    if key not in _CACHE:
        _CACHE[key] = _build_program(with_qk_bias, with_v_bias)
    nc = _CACHE[key]

    # weight layout prep: wT[cin, ab*128 + c] = W[c, cin, a, b];
    # q and k weights packed into one tensor for a single early DMA
    wqT = Wq.transpose(1, 2, 3, 0).reshape(C, 4 * C)
    wkT = Wk.transpose(1, 2, 3, 0).reshape(C, 4 * C)
    wqk = np.ascontiguousarray(np.concatenate([wqT, wkT], axis=1))
    wv = np.ascontiguousarray(Wv.reshape(C, C))

    in_maps = []
    for b in range(B):
        m = {
            "x": np.ascontiguousarray(x[b].reshape(C, HW)),
            "wqk": wqk,
            "wv": wv,
        }
        if with_qk_bias:
            m["bq"] = np.ascontiguousarray(bq.reshape(C, 1))
            m["bk"] = np.ascontiguousarray(bk.reshape(C, 1))
        if with_v_bias:
            m["bv"] = np.ascontiguousarray(bv.reshape(C, 1))
        in_maps.append(m)

    res = run_bass_kernel_spmd(nc, in_maps, list(range(N_CORES)))
    out = np.stack([res.results[i]["out"] for i in range(N_CORES)])
    return out.reshape(B, C, H, W).astype(np.float32)


# revision 18
# speedup vs baseline: 1.0502x; 1.0138x over previous
"""CAM (channel attention) module kernel for Trainium2, 8-core data-parallel.

Reference computation (per sample b):
    q = conv2d(x, Wq, stride2, 2x2) -> [C, 4096]
    k = conv2d(x, Wk, stride2, 2x2) -> [C, 4096]
    v = conv2d(x, Wv, 1x1)          -> [C, 16384]
    E = q @ k^T                      [C, C]
    att = softmax(rowmax(E) - E)   (== softmin over rows)
    out = att @ v -> [C, H, W]

Kernel strategy (one sample per NeuronCore, B=8 over 8 cores):
  - The softmax here is extremely peaked (energy entries span +-200), so
    energy errors are amplified exponentially: q/k need ~16+ mantissa
    bits, which rules out bf16/fp16 single- or 2-pass schemes for the
    convs (2-pass fp16 measures 2.2e-2 rel err vs the 2e-2 gate).
    The cheapest precise scheme is split-bf16: x = xh + xl with
    xh = bf16(x), xl = bf16(x - xh) (~16-bit combined); same for the
    conv weights. conv = Wh@xh + Wh@xl + Wl@xh: 3 full-rate bf16
    passes (bf16 moving operands stream 1 cyc/row vs fp32's 4).
    (A transposed x-stationary formulation would skip the q/k
    transposes, but the PE stationary operand must be a single-stride
    AP and the im2col tap views are not — verified to fail BIR.)
  - Startup: wqk loads first (small, needed by every conv matmul, with
    per-tap bf16 splits so tap 0 is ready ~0.5us after it lands), then
    band 0 in two half-band pieces so the DVE split overlaps the DMA,
    then bands 1-7. Dummy ident matmuls pre-warm the PE clock ramp
    while the input streams in.
  - conv emits pass-major (all xh-matmuls before xl) with q/k
    interleaved, so a band's convs can start before its xl split is
    done; stationary (weight) loads are fully hidden under the streams.
  - conv produces q in [c, n] layout via strided im2col APs from the
    resident xh/xl tiles, then PE-transposes to [n, c] chunks for the
    energy contraction. Transposes + energy chunk matmuls interleave
    into the conv stream (one piece behind) so the PE never idles.
  - energy e = q k^T in native fp32 (exact), accumulated over 32 chunk
    matmuls in one PSUM bank.
  - softmin via one DVE row-min + one ScalarE exp (bias=rowmin,
    scale=-1) with fused accumulated row-sum. Normalization is
    DEFERRED: the unnormalized w = exp(min - E) feeds the value path
    and the final copies scale by 1/z per partition, keeping the
    reciprocal off the serial chain.
  - out = att @ (Wv x + bv) == (1/z) * (w Wv) @ x (+ scaled bias):
    computes M'^T = Wv^T w^T on PE ([128,128]), casts to bf16 once,
    and runs out = Mh@xh against the resident split-x tiles (1 bf16
    pass, one stationary load; the dropped lo terms cost ~2.2e-3 rel
    err total vs the 2e-2 gate). The phase is store-DMA-paced: stores
    start right after each band's first PSUM copy, the first band is
    split 2x and the last 4x across queues to shrink lead-in/tail.
"""

import numpy as np

B, C, H, W = 8, 128, 128, 128
HW = H * W           # 16384
N_CORES = 8
NB = 8               # number of H-bands (16 input rows each)
BAND = HW // NB      # 2048 x columns per band
HALF = BAND // 2     # 1024
QN = (H // 2) * (W // 2)  # 4096 conv output positions
QCHUNK = QN // NB    # 512 conv outputs per band

OUT_PASSES = 1       # bf16 passes in the output matmul (1..2)
N_PREWARM = 20       # PE ramp filler during input DMA
N_SOFTWARM = 14      # PE filler across the softmax serial chain
TP_F32R = False      # fp32r transposes: BIR demands fp32r-rounded
                     # producers, i.e. a real 12-bit rounding -- unsafe
                     # for the 16-bit energy path. Keep fp32.

_CACHE = {}


def _build_program(with_qk_bias: bool, with_v_bias: bool):
    import concourse.tile as tile
    from concourse import bacc, mybir
    from concourse.masks import make_identity

    f32 = mybir.dt.float32
    bf16 = mybir.dt.bfloat16
    Ident = mybir.ActivationFunctionType.Identity
    CopyF = mybir.ActivationFunctionType.Copy
    nc = bacc.Bacc(
        "TRN2", target_bir_lowering=False, debug=False, num_devices=N_CORES)

    x_d = nc.declare_dram_parameter("x", [C, HW], f32, isOutput=False)
    wqk_d = nc.declare_dram_parameter("wqk", [C, 8 * C], f32, isOutput=False)
    wv_d = nc.declare_dram_parameter("wv", [C, C], f32, isOutput=False)
    if with_qk_bias:
        bq_d = nc.declare_dram_parameter("bq", [C, 1], f32, isOutput=False)
        bk_d = nc.declare_dram_parameter("bk", [C, 1], f32, isOutput=False)
    if with_v_bias:
        bv_d = nc.declare_dram_parameter("bv", [C, 1], f32, isOutput=False)
    out_d = nc.declare_dram_parameter("out", [C, HW], f32, isOutput=True)

    with tile.TileContext(nc) as tc:
        with (
            tc.tile_pool(name="const", bufs=1) as const,
            tc.tile_pool(name="xstage", bufs=3) as xstage,
            tc.tile_pool(name="xrp", bufs=1) as xrp,
            tc.tile_pool(name="qkT", bufs=1) as qkT,
            tc.tile_pool(name="stage", bufs=3) as stage,
            tc.tile_pool(name="oout", bufs=3) as oout,
            tc.tile_pool(name="small", bufs=2) as small,
            tc.tile_pool(name="pacc", bufs=5, space="PSUM") as pacc,
            tc.tile_pool(name="ptp", bufs=2, space="PSUM") as ptp,
            tc.tile_pool(name="psm", bufs=1, space="PSUM") as psm,
        ):
            ident = const.tile([128, 128], f32, tag="ident")
            make_identity(nc, ident)

            # Startup: per-core DMA bandwidth is ONE shared resource, so
            # serialize ALL input loads on the sync queue in consumption
            # order: wqk (every conv matmul needs it), band 0 as two
            # half-band pieces (the DVE split pipeline starts after the
            # first 0.5MiB), then the remaining bands.
            wqk_sb = const.tile([C, 8 * C], f32, tag="wqk")
            nc.sync.dma_start(out=wqk_sb, in_=wqk_d[:, :])
            wqT_sb = wqk_sb[:, 0:4 * C]
            wkT_sb = wqk_sb[:, 4 * C:8 * C]
            if with_qk_bias:
                bq_sb = const.tile([C, 1], f32, tag="bq")
                nc.sync.dma_start(out=bq_sb, in_=bq_d[:, :])
                bk_sb = const.tile([C, 1], f32, tag="bk")
                nc.sync.dma_start(out=bk_sb, in_=bk_d[:, :])
            x_sb = []
            x0 = xstage.tile([C, BAND], f32, tag="x", name="x0")
            nc.sync.dma_start(out=x0[:, 0:HALF], in_=x_d[:, 0:HALF])
            nc.sync.dma_start(out=x0[:, HALF:BAND], in_=x_d[:, HALF:BAND])
            x_sb.append(x0)
            for j in range(1, NB):
                t = xstage.tile([C, BAND], f32, tag="x", name=f"x{j}")
                nc.sync.dma_start(out=t, in_=x_d[:, j * BAND:(j + 1) * BAND])
                x_sb.append(t)
            wv_sb = const.tile([C, C], f32, tag="wv")
            nc.sync.dma_start(out=wv_sb, in_=wv_d[:, :])
            if with_v_bias:
                bv_sb = const.tile([C, 1], f32, tag="bv")
                nc.sync.dma_start(out=bv_sb, in_=bv_d[:, :])

            # PE clock pre-warm: the tensor engine ramps to full clock
            # only after ~3us of continuous work, and the first real
            # matmul can't start until weights + band 0 have landed.
            # Fill the wait with throwaway ident matmuls.
            for dw in range(N_PREWARM):
                ws = pacc.tile([128, 128], f32, tag="acc", name=f"pw{dw}")
                nc.tensor.matmul(ws, lhsT=ident, rhs=ident,
                                 start=True, stop=True)

            # split the conv weights into bf16 hi/lo PER TAP, tap-major
            # q-then-k, so the first conv matmul's weights are ready
            # right after wqk lands (hi cast on ScalarE, lo sub on
            # GpSimd; DVE stays free for the x splits)
            wqh = const.tile([C, 4 * C], bf16, tag="wqh")
            wql = const.tile([C, 4 * C], bf16, tag="wql")
            wkh = const.tile([C, 4 * C], bf16, tag="wkh")
            wkl = const.tile([C, 4 * C], bf16, tag="wkl")
            for ab in range(4):
                s = slice(ab * C, (ab + 1) * C)
                for (hi, lo, src) in ((wqh, wql, wqT_sb), (wkh, wkl, wkT_sb)):
                    nc.scalar.activation(out=hi[:, s], in_=src[:, s],
                                         func=CopyF, bias=0.0, scale=1.0)
                    nc.gpsimd.tensor_tensor(
                        out=lo[:, s], in0=src[:, s], in1=hi[:, s],
                        op=mybir.AluOpType.subtract)

            xh_sb = [xrp.tile([C, BAND], bf16, tag=f"xh{j}", name=f"xh{j}")
                     for j in range(NB)]
            xl_sb = [xrp.tile([C, BAND], bf16, tag=f"xl{j}", name=f"xl{j}")
                     for j in range(NB)]

            def split_cols(j, lo, hi):
                nc.vector.tensor_copy(xh_sb[j][:, lo:hi], x_sb[j][:, lo:hi])
                nc.vector.tensor_tensor(
                    out=xl_sb[j][:, lo:hi], in0=x_sb[j][:, lo:hi],
                    in1=xh_sb[j][:, lo:hi], op=mybir.AluOpType.subtract)

            qT = [qkT.tile([128, QCHUNK], f32, tag=f"qT{j}", name=f"qT{j}")
                  for j in range(NB)]
            kT = [qkT.tile([128, QCHUNK], f32, tag=f"kT{j}", name=f"kT{j}")
                  for j in range(NB)]

            # energy accumulator lives across the whole conv phase:
            # energy chunk matmuls are interleaved into the conv stream
            # so the PE never sits in a low-duty phase.
            E = psm.tile([128, 128], f32, tag="E")
            e_idx = [0]

            def conv_piece(j, lo, ncols):
                """q and k convs for x columns [lo, lo+ncols) of band j,
                interleaved pass-major: all xh matmuls first so the
                piece's xl split can finish in their shadow. Returns the
                two PSUM accumulators ([128, ncols//4] each)."""
                i_cnt = ncols // 256
                nout = ncols // 4
                xh_v = xh_sb[j][:, lo:lo + ncols].rearrange(
                    "p (i a w b) -> p i a w b", i=i_cnt, a=2, w=64, b=2)
                xl_v = xl_sb[j][:, lo:lo + ncols].rearrange(
                    "p (i a w b) -> p i a w b", i=i_cnt, a=2, w=64, b=2)
                acc_q = pacc.tile([128, 512], f32, tag="acc",
                                  name=f"aq{j}_{lo}")
                acc_k = pacc.tile([128, 512], f32, tag="acc",
                                  name=f"ak{j}_{lo}")
                n_mm = 0
                for wq_s, wk_s, xv in ((wqh, wkh, xh_v), (wqh, wkh, xl_v),
                                       (wql, wkl, xh_v)):
                    for ab in range(4):
                        a, bb = ab // 2, ab % 2
                        rhs = xv[:, :, a, :, bb]
                        nc.tensor.matmul(
                            acc_q[:, 0:nout],
                            lhsT=wq_s[:, ab * C:(ab + 1) * C], rhs=rhs,
                            start=(n_mm == 0), stop=(n_mm == 11))
                        nc.tensor.matmul(
                            acc_k[:, 0:nout],
                            lhsT=wk_s[:, ab * C:(ab + 1) * C], rhs=rhs,
                            start=(n_mm == 0), stop=(n_mm == 11))
                        n_mm += 1
                return acc_q, acc_k

            f32r = mybir.dt.float32r

            def emit_tp_energy(j, qoff, nout, qc, kc):
                for T_out, src in ((qT[j], qc), (kT[j], kc)):
                    tp = ptp.tile([128, 512], f32, tag="tp")
                    for t in range(nout // 128):
                        o_ap = tp[:, t * 128:(t + 1) * 128]
                        s_ap = src[:, t * 128:(t + 1) * 128]
                        if TP_F32R:
                            # pure permutation; fp32r moves the same 32
                            # bits at 1.5 cyc/row instead of fp32's 2
                            nc.tensor.transpose(
                                o_ap.bitcast(f32r), s_ap.bitcast(f32r),
                                ident[:, :].bitcast(f32r))
                        else:
                            nc.tensor.transpose(o_ap, s_ap, ident)
                    nc.scalar.activation(
                        out=T_out[:, qoff:qoff + nout], in_=tp[:, 0:nout],
                        func=CopyF, bias=0.0, scale=1.0)
                for t in range(nout // 128):
                    o = qoff + t * 128
                    nc.tensor.matmul(
                        E,
                        lhsT=qT[j][:, o:o + 128],
                        rhs=kT[j][:, o:o + 128],
                        start=(e_idx[0] == 0), stop=(e_idx[0] == NB * 4 - 1))
                    e_idx[0] += 1

            pieces = [(0, 0, HALF), (0, HALF, HALF)]
            pieces += [(j, 0, BAND) for j in range(1, NB)]

            pend = None
            for (j, lo, ncols) in pieces:
                split_cols(j, lo, lo + ncols)
                acc_q, acc_k = conv_piece(j, lo, ncols)
                nout = ncols // 4
                qc = stage.tile([128, 512], f32, tag="qchunk",
                                name=f"qc{j}_{lo}")
                kc = stage.tile([128, 512], f32, tag="kchunk",
                                name=f"kc{j}_{lo}")
                if with_qk_bias:
                    nc.scalar.activation(out=qc[:, 0:nout],
                                         in_=acc_q[:, 0:nout], func=Ident,
                                         bias=bq_sb[:, 0:1], scale=1.0)
                    nc.scalar.activation(out=kc[:, 0:nout],
                                         in_=acc_k[:, 0:nout], func=Ident,
                                         bias=bk_sb[:, 0:1], scale=1.0)
                else:
                    nc.scalar.activation(out=qc[:, 0:nout],
                                         in_=acc_q[:, 0:nout], func=CopyF,
                                         bias=0.0, scale=1.0)
                    nc.scalar.activation(out=kc[:, 0:nout],
                                         in_=acc_k[:, 0:nout], func=CopyF,
                                         bias=0.0, scale=1.0)
                # transposes + energy one piece behind
                if pend is not None:
                    emit_tp_energy(*pend)
                pend = (j, lo // 4, nout, qc, kc)
            emit_tp_energy(*pend)

            # keep the PE busy through the softmax serial chain so the
            # clock gate doesn't re-throttle before the output matmuls
            # (results unused; inputs are long since ready)
            for dw in range(N_SOFTWARM):
                scratch = pacc.tile([128, 256], f32, tag="acc",
                                    name=f"warm{dw}")
                nc.tensor.matmul(
                    scratch, lhsT=wqh[:, 0:128],
                    rhs=xh_sb[0][:, 0:256],
                    start=True, stop=True)

            # softmin over rows: att = exp(min-E) / z. Normalizing att
            # up front keeps every out-phase PSUM->SBUF copy a PLAIN
            # copy (DVE runs 2x faster than with a fused scale), which
            # is what paces the store phase.
            mmin = small.tile([128, 1], f32, tag="mmin")
            nc.vector.tensor_reduce(
                out=mmin, in_=E, axis=mybir.AxisListType.X,
                op=mybir.AluOpType.min)
            w_sb = small.tile([128, 128], f32, tag="w")
            zsum = small.tile([128, 1], f32, tag="z")
            nc.scalar.activation(
                out=w_sb, in_=E, func=mybir.ActivationFunctionType.Exp,
                bias=mmin[:, 0:1], scale=-1.0, accum_out=zsum[:, 0:1])
            rz = small.tile([128, 1], f32, tag="rz")
            nc.vector.reciprocal(rz, zsum)
            att = small.tile([128, 128], f32, tag="att")
            nc.vector.tensor_scalar_mul(att, w_sb, rz[:, 0:1])

            attT_p = psm.tile([128, 128], f32, tag="E")
            nc.tensor.transpose(attT_p, att, ident)
            attT = small.tile([128, 128], f32, tag="attT")
            nc.vector.tensor_copy(attT, attT_p)

            # M^T[c2, c] = sum_d Wv[d, c2] attT[d, c], cast to bf16
            # once; the dropped lo terms cost ~2.2e-3 rel err.
            MT_p = psm.tile([128, 128], f32, tag="E")
            nc.tensor.matmul(MT_p, lhsT=wv_sb, rhs=attT,
                             start=True, stop=True)
            Mh = small.tile([128, 128], bf16, tag="Mh")
            nc.vector.tensor_copy(Mh, MT_p)

            if with_v_bias:
                abv_p = psm.tile([128, 1], f32, tag="E")
                nc.tensor.matmul(abv_p, lhsT=attT, rhs=bv_sb[:, 0:1],
                                 start=True, stop=True)
                abv = small.tile([128, 1], f32, tag="abv")
                nc.vector.tensor_copy(abv, abv_p)

            # out[c, n] = sum_c2 M[c, c2] x[c2, n] (+ bias) via bf16; one
            # stationary load of Mh covers everything, PSUM accumulators
            # rotate through the pool. The phase must be store-DMA-paced
            # (~2.9us/band), so the per-band copies spread 3:1 over
            # DVE/ScalarE (plain copies, ~1.1us + 0.7us) and the store
            # descriptors issue from the otherwise-idle sync/gpsimd
            # queues. The first band is split 2x and the last 4x to
            # shrink lead-in/tail.
            out_dma_engines = [nc.sync, nc.gpsimd]
            for j in range(NB):
                o_band = oout.tile([128, BAND], f32, tag="oband")
                o_ps = [pacc.tile([128, 512], f32, tag="acc",
                                  name=f"ops{j}_{s}")
                        for s in range(4)]
                for s in range(4):
                    nc.tensor.matmul(
                        o_ps[s], lhsT=Mh,
                        rhs=xh_sb[j][:, s * 512:(s + 1) * 512],
                        start=True, stop=(OUT_PASSES == 1))
                if OUT_PASSES >= 2:
                    for s in range(4):
                        nc.tensor.matmul(
                            o_ps[s], lhsT=Mh,
                            rhs=xl_sb[j][:, s * 512:(s + 1) * 512],
                            start=False, stop=True)
                for s in range(4):
                    dst = o_band[:, s * 512:(s + 1) * 512]
                    if with_v_bias:
                        nc.scalar.activation(
                            out=dst, in_=o_ps[s], func=Ident,
                            bias=abv[:, 0:1], scale=1.0)
                    elif s == 3:
                        nc.scalar.activation(out=dst, in_=o_ps[s], func=CopyF,
                                             bias=0.0, scale=1.0)
                    else:
                        nc.vector.tensor_copy(dst, o_ps[s])
                n_pieces = 4 if j == NB - 1 else (2 if j == 0 else 1)
                psz = BAND // n_pieces
                for h in range(n_pieces):
                    off = j * BAND + h * psz
                    out_dma_engines[(j + h) % 2].dma_start(
                        out=out_d[:, off:off + psz],
                        in_=o_band[:, h * psz:(h + 1) * psz])

    nc.compile()
    return nc


def kernel(x, Wq, bq, Wk, bk, Wv, bv):
    from concourse.bass_utils import run_bass_kernel_spmd

    x = np.ascontiguousarray(np.asarray(x, dtype=np.float32))
    Wq = np.asarray(Wq, dtype=np.float32)
    Wk = np.asarray(Wk, dtype=np.float32)
    Wv = np.asarray(Wv, dtype=np.float32)
    bq = np.asarray(bq, dtype=np.float32)
    bk = np.asarray(bk, dtype=np.float32)
    bv = np.asarray(bv, dtype=np.float32)

    with_qk_bias = bool(np.any(bq) or np.any(bk))
    with_v_bias = bool(np.any(bv))

    key = (with_qk_bias, with_v_bias)
    if key not in _CACHE:
        _CACHE[key] = _build_program(with_qk_bias, with_v_bias)
    nc = _CACHE[key]

    # weight layout prep: wT[cin, ab*128 + c] = W[c, cin, a, b];
    # q and k weights packed into one tensor for a single early DMA
    wqT = Wq.transpose(1, 2, 3, 0).reshape(C, 4 * C)
    wkT = Wk.transpose(1, 2, 3, 0).reshape(C, 4 * C)
    wqk = np.ascontiguousarray(np.concatenate([wqT, wkT], axis=1))
    wv = np.ascontiguousarray(Wv.reshape(C, C))

    in_maps = []
    for b in range(B):
        m = {
            "x": np.ascontiguousarray(x[b].reshape(C, HW)),
            "wqk": wqk,
            "wv": wv,
        }
        if with_qk_bias:
            m["bq"] = np.ascontiguousarray(bq.reshape(C, 1))
            m["bk"] = np.ascontiguousarray(bk.reshape(C, 1))
        if with_v_bias:
            m["bv"] = np.ascontiguousarray(bv.reshape(C, 1))
        in_maps.append(m)

    res = run_bass_kernel_spmd(nc, in_maps, list(range(N_CORES)))
    out = np.stack([res.results[i]["out"] for i in range(N_CORES)])
    return out.reshape(B, C, H, W).astype(np.float32)
